# revision 8
# baseline (speedup 1.0000x reference)
"""LFQ (lookup-free quantization) Trainium2 kernel, 8-core SPMD.

Math: the 16384-code softmax factorizes over the 14 sign dims into a
product of Bernoullis q_d = sigmoid(400*h_d).  Splitting dims 0-6 / 7-13
makes each token's 16384-prob vector the outer product u (x) w of two
128-vectors, so avg_prob = (1/T) sum_t u_t w_t^T is a single PE-matmul
accumulation and the per-sample entropy is the analytic sum of 14
binary entropies.  Only the codebook entropy needs the reference's
clip(p, 1e-5) semantics, applied once to the final [128,128] avg_prob.
Cross-core reduction (avg_prob + scalar partials) is one packed
AllReduce of [128,130] f32.
"""

import numpy as np

DIM = 14
B, N = 4, 2048
TOK = B * N                 # 8192 tokens
NCORES = 8
TPC = TOK // NCORES         # 1024 tokens per core
G = TPC // 128              # 8 groups of 128 tokens (partition dim)
INV_TEMP = 100.0
EPS = 1e-5
HCLAMP = -0.215             # exp(-400*h) <= e^86, finite in f32
QCLAMP = 1.2e-38            # keep Ln input normal
ENT_W = 0.1
COMMIT_W = 0.25

_cache = {}


def _build():
    import concourse.bass as bass
    import concourse.bacc as bacc
    import concourse.tile as tile
    from concourse import mybir

    f32 = mybir.dt.float32
    i32 = mybir.dt.int32
    ALU = mybir.AluOpType
    ACT = mybir.ActivationFunctionType
    AX = mybir.AxisListType

    nc = bacc.Bacc("TRN2", target_bir_lowering=False, debug=False,
                   num_devices=NCORES)

    xT = nc.dram_tensor("xT", [15, TPC], f32, kind="ExternalInput")
    wi = nc.dram_tensor("wi", [15, DIM], f32, kind="ExternalInput")
    wo = nc.dram_tensor("wo", [15, DIM], f32, kind="ExternalInput")
    bm = nc.dram_tensor("bm", [DIM, 1], f32, kind="ExternalInput")
    out_d = nc.dram_tensor("out_sh", [TPC, DIM], f32, kind="ExternalOutput")
    idx_d = nc.dram_tensor("idx", [TPC], i32, kind="ExternalOutput")
    aux_d = nc.dram_tensor("aux", [1], f32, kind="ExternalOutput")

    def bcast_last(v, rep):
        # append broadcast dim: read each element `rep` times consecutively
        return bass.AP(tensor=v.tensor, offset=v.offset, ap=[*v.ap, [0, rep]])

    def bcast_mid(v, rep):
        # insert broadcast dim before last: repeat the last-dim block `rep` times
        return bass.AP(tensor=v.tensor, offset=v.offset,
                       ap=[*v.ap[:-1], [0, rep], v.ap[-1]])

    def split_last(v, inner):
        # reshape last dim [S] -> [S//inner, inner]
        st, cnt = v.ap[-1]
        assert st == 1 and cnt % inner == 0
        return bass.AP(tensor=v.tensor, offset=v.offset,
                       ap=[*v.ap[:-1], [inner, cnt // inner], [1, inner]])

    with tile.TileContext(nc) as tc:
        with (
            tc.tile_pool(name="const", bufs=1) as const,
            tc.tile_pool(name="sb", bufs=1) as sb,
            tc.tile_pool(name="psum", bufs=1, space="PSUM") as pp,
            tc.tile_pool(name="dram", bufs=1, space="DRAM") as dp,
        ):
            xT_sb = const.tile([15, TPC], f32)
            wi_sb = const.tile([15, DIM], f32)
            wo_sb = const.tile([15, DIM], f32)
            bm_sb = const.tile([DIM, 1], f32)
            nc.gpsimd.dma_start(out=xT_sb[:], in_=xT[:])
            nc.gpsimd.dma_start(out=wi_sb[:], in_=wi[:])
            nc.gpsimd.dma_start(out=wo_sb[:], in_=wo[:])
            nc.gpsimd.dma_start(out=bm_sb[:], in_=bm[:])

            # ---- h in token-partition layout: [128, G, 14] ----
            h_ps = pp.tile([128, G, DIM], f32, tag="psA")
            for g in range(G):
                nc.tensor.matmul(h_ps[:, g, :], xT_sb[:, g * 128:(g + 1) * 128],
                                 wi_sb[:], start=True, stop=True)

            # ---- hT in feature-partition layout: [14, 1024] ----
            hT_ps = pp.tile([DIM, TPC], f32)
            for k in range(TPC // 512):
                nc.tensor.matmul(hT_ps[:, k * 512:(k + 1) * 512], wi_sb[:],
                                 xT_sb[:, k * 512:(k + 1) * 512],
                                 start=True, stop=True)

            # ---- Bernoulli factors: F[p, g, d, {1-q, q}] ----
            FW = G * DIM * 2
            F = sb.tile([128, G, DIM, 2], f32)
            hc = sb.tile([128, G * DIM], f32)
            e = sb.tile([128, G * DIM], f32)
            se = sb.tile([128, G * DIM], f32)
            h_flat = h_ps[:].rearrange("p g d -> p (g d)")
            F_q = bass.AP(tensor=F.tensor, offset=F.offset + 1,
                          ap=[F[:].ap[0], [2, G * DIM]])       # F[...,1] slots
            F_qm = bass.AP(tensor=F.tensor, offset=F.offset,
                           ap=[F[:].ap[0], [2, G * DIM]])      # F[...,0] slots
            nc.vector.tensor_scalar_max(hc[:], h_flat, HCLAMP)
            nc.scalar.activation(e[:], hc[:], ACT.Exp, scale=-400.0)
            nc.vector.tensor_scalar_add(se[:], e[:], 1.0)
            nc.vector.reciprocal(F_q, se[:])                   # q = 1/(1+e)
            nc.vector.tensor_mul(F_qm, e[:], F_q)              # 1-q = e*q

            # ---- per-token entropy sum (analytic, sign-negated) ----
            Fc = sb.tile([128, FW], f32)
            lnF = sb.tile([128, FW], f32)
            xlx = sb.tile([128, FW], f32)
            pack = sb.tile([128, 2], f32)       # col0: ent, col1: commit
            F_flat = F[:].rearrange("p g d b -> p (g d b)")
            nc.vector.tensor_scalar_max(Fc[:], F_flat, QCLAMP)
            nc.scalar.activation(lnF[:], Fc[:], ACT.Ln)
            nc.vector.tensor_mul(xlx[:], F_flat, lnF[:])
            nc.vector.tensor_reduce(pack[:, 0:1], xlx[:], axis=AX.X,
                                    op=ALU.add)

            # ---- commitment loss partial: sum (|h|-1)^2 ----
            ab = sb.tile([128, G * DIM], f32)
            am = sb.tile([128, G * DIM], f32)
            sq = sb.tile([128, G * DIM], f32)
            nc.scalar.activation(ab[:], h_flat, ACT.Abs)
            nc.vector.tensor_scalar_add(am[:], ab[:], -1.0)
            nc.scalar.activation(sq[:], am[:], ACT.Square,
                                 accum_out=pack[:, 1:2])

            # ---- Kronecker doubling: u (dims 0-6), w (dims 7-13) ----
            def build(d0, dst):
                cur = F[:, :, d0, :]                       # [128, G, 2]
                for lev in range(1, 7):
                    S = 2 << lev                           # 4, 8, ..., 128
                    nxt = dst if S == 128 else sb.tile(
                        [128, G, S], f32, tag=f"kb{d0}_{lev & 1}")
                    o = nxt[:, :, 0:S] if S != 128 else nxt[:]
                    nc.vector.tensor_tensor(
                        out=split_last(o, 2),
                        in0=bcast_last(cur, 2),
                        in1=bcast_mid(F[:, :, d0 + lev, :], S // 2),
                        op=ALU.mult)
                    cur = nxt[:, :, 0:S]
                return dst

            u = sb.tile([128, G, 128], f32)
            w = sb.tile([128, G, 128], f32)
            build(0, u)
            build(7, w)

            # ---- avg_prob partial: sum_t u_t w_t^T on PE ----
            avg_ps = pp.tile([128, 128], f32)
            for g in range(G):
                nc.tensor.matmul(avg_ps[:], u[:, g, :], w[:, g, :],
                                 start=(g == 0), stop=(g == G - 1))

            # ---- indices + quantized output ----
            bits = sb.tile([DIM, TPC], f32)
            quant = sb.tile([15, TPC], f32)
            nc.vector.tensor_scalar(out=bits[:], in0=hT_ps[:], scalar1=0.0,
                                    scalar2=None, op0=ALU.is_gt)
            # partition writes must start on a quad boundary: fill the whole
            # tile with the bias row's 1.0, then overwrite rows 0-13
            nc.vector.memset(quant[:], 1.0)
            nc.vector.tensor_scalar(out=quant[0:DIM, :], in0=bits[:],
                                    scalar1=2.0, scalar2=-1.0,
                                    op0=ALU.mult, op1=ALU.add)

            idx_ps = pp.tile([1, TPC], f32)
            for k in range(TPC // 512):
                nc.tensor.matmul(idx_ps[:, k * 512:(k + 1) * 512], bm_sb[:],
                                 bits[:, k * 512:(k + 1) * 512],
                                 start=True, stop=True)
            idx_sb = sb.tile([1, TPC], i32)
            nc.vector.tensor_copy(out=idx_sb[:], in_=idx_ps[:])
            nc.gpsimd.dma_start(out=idx_d[:].rearrange("(a t) -> a t", a=1),
                              in_=idx_sb[:])

            out_ps = pp.tile([128, G, DIM], f32, tag="psB")
            for g in range(G):
                nc.tensor.matmul(out_ps[:, g, :],
                                 quant[:, g * 128:(g + 1) * 128],
                                 wo_sb[:], start=True, stop=True)
            out_sb = sb.tile([128, G, DIM], f32)
            nc.scalar.copy(out_sb[:], out_ps[:])
            # DRAM out[t, o], t = g*128 + p  ->  p stride 14, g stride 1792
            out_view = bass.AP(tensor=out_d[:].tensor, offset=0,
                               ap=[[DIM, 128], [128 * DIM, G], [1, DIM]])
            nc.gpsimd.dma_start(out=out_view, in_=out_sb[:])

            # ---- pack AllReduce payload [128, 130] ----
            ones = const.tile([128, 1], f32)
            nc.vector.memset(ones[:], 1.0)
            red_ps = pp.tile([1, 2], f32, tag="psA")
            nc.tensor.matmul(red_ps[:], ones[:], pack[:], start=True, stop=True)
            ar = sb.tile([128, 130], f32)
            nc.scalar.copy(ar[:, 0:128], avg_ps[:])
            nc.vector.memset(ar[:, 128:130], 0.0)
            nc.vector.tensor_copy(out=ar[0:1, 128:130], in_=red_ps[:])

            cc_in = dp.tile([128, 130], f32)
            cc_out = dp.tile([128, 130], f32, addr_space="Shared")
            nc.gpsimd.dma_start(out=cc_in[:], in_=ar[:])
            nc.gpsimd.collective_compute(
                "AllReduce", ALU.add,
                replica_groups=[list(range(NCORES))],
                ins=[cc_in[:].opt()], outs=[cc_out[:].opt()])
            ar2 = sb.tile([128, 130], f32)
            nc.gpsimd.dma_start(out=ar2[:], in_=cc_out[:])

            # ---- codebook entropy with clip semantics ----
            As = sb.tile([128, 128], f32)
            Ac = sb.tile([128, 128], f32)
            lnA = sb.tile([128, 128], f32)
            scr = sb.tile([128, 128], f32)
            cb_p = sb.tile([128, 1], f32)
            nc.vector.tensor_scalar_mul(As[:], ar2[:, 0:128], 1.0 / TOK)
            nc.vector.tensor_scalar_max(Ac[:], As[:], EPS)
            nc.scalar.activation(lnA[:], Ac[:], ACT.Ln)
            nc.vector.tensor_mul(scr[:], As[:], lnA[:])
            nc.vector.tensor_reduce(cb_p[:], scr[:], axis=AX.X, op=ALU.add)

            # ---- final aux scalar ----
            cb_ps = pp.tile([1, 1], f32, tag="psB")
            nc.tensor.matmul(cb_ps[:], ones[:], cb_p[:], start=True, stop=True)
            fin = sb.tile([1, 3], f32)
            aux_sb = sb.tile([1, 1], f32)
            nc.vector.tensor_scalar_mul(fin[0:1, 0:1], cb_ps[:], ENT_W)
            nc.vector.tensor_scalar_mul(fin[0:1, 1:2], ar2[0:1, 128:129],
                                        -ENT_W / TOK)
            nc.vector.tensor_scalar_mul(fin[0:1, 2:3], ar2[0:1, 129:130],
                                        COMMIT_W / (TOK * DIM))
            nc.vector.tensor_reduce(out=aux_sb[:], in_=fin[0:1, 0:3],
                                    axis=AX.X, op=ALU.add)
            nc.gpsimd.dma_start(out=aux_d[:].rearrange("(a t) -> a t", a=1),
                              in_=aux_sb[:])

    nc.compile()
    return nc


def _get_nc():
    if "nc" not in _cache:
        _cache["nc"] = _build()
    return _cache["nc"]


def _prep_inputs(x, W_in, b_in, W_out, b_out):
    x = np.ascontiguousarray(np.asarray(x, np.float32)).reshape(TOK, DIM)
    wi = np.concatenate([np.asarray(W_in, np.float32).T,
                         np.asarray(b_in, np.float32)[None, :]], 0)
    wo = np.concatenate([np.asarray(W_out, np.float32).T,
                         np.asarray(b_out, np.float32)[None, :]], 0)
    bm = (2.0 ** np.arange(DIM - 1, -1, -1, dtype=np.float32))[:, None]
    wi = np.ascontiguousarray(wi)
    wo = np.ascontiguousarray(wo)
    in_maps = []
    for c in range(NCORES):
        sh = x[c * TPC:(c + 1) * TPC]                      # [1024, 14]
        xTa = np.empty((15, TPC), np.float32)
        xTa[:DIM] = sh.T
        xTa[DIM] = 1.0
        in_maps.append({"xT": np.ascontiguousarray(xTa), "wi": wi,
                        "wo": wo, "bm": bm})
    return in_maps


def _run(in_maps, trace=False):
    from concourse.bass_utils import run_bass_kernel_spmd
    nc = _get_nc()
    return run_bass_kernel_spmd(nc, in_maps, list(range(NCORES)), trace=trace)


def kernel(x, W_in, b_in, W_out, b_out):
    in_maps = _prep_inputs(x, W_in, b_in, W_out, b_out)
    res = _run(in_maps).results
    out = np.concatenate([res[c]["out_sh"] for c in range(NCORES)], 0)
    idx = np.concatenate([res[c]["idx"] for c in range(NCORES)], 0)
    aux = np.float32(res[0]["aux"][0])
    return (out.reshape(B, N, DIM),
            idx.reshape(B, N).astype(np.int32), aux)
